# revision 1
# baseline (speedup 1.0000x reference)
"""Multi-head self-attention on 8 TRN2 NeuronCores, batch-data-parallel.

Problem (hardcoded): inputs (8, 1024, 1024) f32, Wq/Wk/Wv (1024, 1024) f32,
heads=16, head_dim=64. out[b,q,h,v] = softmax(Q K^T / 8) V per head.

Sharding: batch b -> core b (8 cores, one batch element each, weights
replicated). No collectives needed.

Per-core dataflow (all matmuls bf16, PSUM fp32):
  xT = x^T arranged on host                  (d on partitions)
  QT[p] = Wq[:,chunk].T @ xT                 ((head*kdim) on partitions)
  KT[p] likewise; V = xT.T @ Wv              (m on partitions, natural)
  scoresT[m,q] = KT.T @ QT per head          (two heads row-packed, K=64 each)
  attnT = exp(scoresT / 8)                   (ScalarE, PSUM -> SBUF bf16)
  outT[v,q] (+ sums row) = [V|1].T @ attnT
  out = transpose(outT) / sums               (xbar DMA transpose + DVE bcast-mul)

The main loop is a software pipeline over head pairs: in phase p the PE
interleaves, per m-chunk step s: scores(p) matmuls, outT(p-1) accumulation,
QT/KT(p+1) projection.  Prologue: inputs stream in chunk-wise so pair-0
projection chains overlap the DMA.  Tail: out(6) is compressed into the
first half of phase 7 and out(7) trails scores(7) chunk-by-chunk, so the
post-scores tail is only the last chains + transpose + normalize + DMA.
"""

import numpy as np

import concourse.bass as bass
import concourse.mybir as mybir
from concourse import bacc
from concourse.tile import TileContext
from concourse.bass_utils import run_bass_kernel_spmd
from contextlib import ExitStack

F32 = mybir.dt.float32
BF16 = mybir.dt.bfloat16

B, W, D = 8, 1024, 1024
H, DK = 16, 64
P = 128
NT = W // P        # 8 partition tiles along q / d / m
NPAIR = H // 2     # 8 head pairs; pair p = heads (2p, 2p+1)
SCALE = float(DK) ** -0.5


def build_nc():
    import time as _time

    _tb = _time.time()
    print("[kernel] building bass graph...", flush=True)
    nc = bacc.Bacc("TRN2", target_bir_lowering=False, debug=False, num_devices=B)
    # Inputs arrive pre-cast to bf16 and pre-arranged on the host:
    # xt[pp, dt, q] = x[q, 128*dt+pp]; w*[pp, dt, n] = W[128*dt+pp, n].
    xt_d = nc.dram_tensor("xt", [P, NT, W], BF16, kind="ExternalInput").ap()
    # wq/wk are pair-major: [pp, pair, d_tile, col] so each projection lhsT
    # block and the priority DMA slice for pair 0 are contiguous.
    wq_d = nc.dram_tensor("wq", [P, NPAIR, NT, P], BF16, kind="ExternalInput").ap()
    wk_d = nc.dram_tensor("wk", [P, NPAIR, NT, P], BF16, kind="ExternalInput").ap()
    wv_d = nc.dram_tensor("wv", [P, NT, H * DK], BF16, kind="ExternalInput").ap()
    out_d = nc.dram_tensor("out", [W, H * DK], F32, kind="ExternalOutput").ap()

    with TileContext(nc) as tc, ExitStack() as ctx:
        big = ctx.enter_context(tc.tile_pool(name="big", bufs=1))
        xT = big.tile([P, NT, W], BF16)           # [d_in_tile, d_tile, q]
        wq_sb = big.tile([P, NPAIR, NT, P], BF16)  # [d_in_tile, pair, d_tile, col]
        wk_sb = big.tile([P, NPAIR, NT, P], BF16)
        wv_sb = big.tile([P, NT, H * DK], BF16)   # [d_in_tile, d_tile, hv]
        vo = big.tile([P, NT, H, DK + 1], BF16)   # [m_in_tile, m_tile, head, v|1]

        # PSUM budget (8 banks): psS 2x(128,1024)f32 = 4 (scores double-buffer
        # against the exp), psP 2x(128,512)f32 = 2 (proj chains ping-pong so
        # the psum->sbuf copy never blocks the next chain), psO 2x(128,512)
        # = 2 (out chains + V half-chains ping-pong likewise).
        psS = ctx.enter_context(tc.tile_pool(name="psS", bufs=2, space="PSUM"))
        psP = ctx.enter_context(tc.tile_pool(name="psP", bufs=2, space="PSUM"))
        psO = ctx.enter_context(tc.tile_pool(name="psO", bufs=2, space="PSUM"))

        qk_pool = ctx.enter_context(tc.tile_pool(name="qk", bufs=2))
        attn_pool = ctx.enter_context(tc.tile_pool(name="attn", bufs=2))
        ot_pool = ctx.enter_context(tc.tile_pool(name="ot", bufs=4))
        ott_pool = ctx.enter_context(tc.tile_pool(name="ott", bufs=4))
        outp_pool = ctx.enter_context(tc.tile_pool(name="outp", bufs=3))
        small_pool = ctx.enter_context(tc.tile_pool(name="small", bufs=4))

        out_view = out_d.rearrange("(t r) n -> r t n", r=P)

        qt_tiles, kt_tiles, attn_tiles, ot_tiles, outp_tiles = {}, {}, {}, {}, {}
        ott_tiles = {}

        proj_state = {}

        def emit_proj_steps(pp, s):
            """Projection work for next pair pp at step s. Each (tensor, nh)
            chain of 8 matmuls is split 4+4 across two steps so score matmuls
            (which feed ScalarE) are never queued behind a full chain."""
            if pp > NPAIR - 1:
                return
            if s == 0:
                qt_tiles[pp] = qk_pool.tile([P, W], BF16, tag="qt", name=f"qt{pp}")
                kt_tiles[pp] = qk_pool.tile([P, W], BF16, tag="kt", name=f"kt{pp}")
            which = s // 2  # 0: QTnh0, 1: KTnh0, 2: QTnh1, 3: KTnh1
            w_sb, dst = [
                (wq_sb, qt_tiles[pp]),
                (wk_sb, kt_tiles[pp]),
                (wq_sb, qt_tiles[pp]),
                (wk_sb, kt_tiles[pp]),
            ][which]
            nh = which // 2
            if s % 2 == 0:
                ps = psP.tile([P, 512], F32, tag="psP", name=f"pp{pp}_{which}")
                proj_state["ps"] = ps
                kds = range(0, 4)
            else:
                ps = proj_state["ps"]
                kds = range(4, NT)
            for kd in kds:
                nc.tensor.matmul(
                    ps[:],
                    lhsT=w_sb[:, pp, kd, :],
                    rhs=xT[:, kd, 512 * nh : 512 * (nh + 1)],
                    start=(kd == 0),
                    stop=(kd == NT - 1),
                )
            if s % 2 == 1:
                nc.vector.tensor_copy(
                    out=dst[:, 512 * nh : 512 * (nh + 1)], in_=ps[:]
                )

        def emit_scores_step(p, s):
            qt_t, kt_t, attnT = qt_tiles[p], kt_tiles[p], attn_tiles[p]
            for hh in range(2):
                ps = psS.tile([P, W], F32, tag="psS")
                lo, hi = DK * hh, DK * (hh + 1)
                for nh in range(2):
                    nc.tensor.matmul(
                        ps[:, 512 * nh : 512 * (nh + 1)],
                        lhsT=kt_t[lo:hi, P * s : P * (s + 1)],
                        rhs=qt_t[lo:hi, 512 * nh : 512 * (nh + 1)],
                        start=True,
                        stop=True,
                        tile_position=(DK * hh, 0),
                    )
                nc.scalar.activation(
                    attnT[:, s, hh, :],
                    ps[:],
                    mybir.ActivationFunctionType.Exp,
                    scale=SCALE,
                )

        out_state = {}

        def emit_out_chain_piece(p, hh, nh, cs, pool, tag):
            """Emit matmuls cs of out-chain (p, hh, nh); allocate psum at c=0,
            copy+maybe-transpose at c=7."""
            attnT = attn_tiles[p]
            h = 2 * p + hh
            key = (p, hh, nh)
            if cs[0] == 0:
                out_state[key] = pool.tile(
                    [DK + 1, 512], F32, tag=tag, name=f"psO{p}_{hh}_{nh}"
                )
            ps_o = out_state[key]
            for c in cs:
                nc.tensor.matmul(
                    ps_o[:],
                    lhsT=vo[:, c, h, :],
                    rhs=attnT[:, c, hh, 512 * nh : 512 * (nh + 1)],
                    start=(c == 0),
                    stop=(c == NT - 1),
                )
            if cs[-1] == NT - 1:
                if nh == 0:
                    # 80 partitions so the xbar transpose DMA constraint
                    # (mult of 16) holds; rows 65-79 are never read back.
                    ot_tiles[(p, hh)] = ot_pool.tile(
                        [80, W], BF16, tag="ot", name=f"ot{p}_{hh}"
                    )
                    ott_tiles[(p, hh)] = ott_pool.tile(
                        [P, NT, 80], BF16, tag="ott", name=f"ott{p}_{hh}"
                    )
                oT = ot_tiles[(p, hh)]
                nc.vector.tensor_copy(
                    out=oT[0 : DK + 1, 512 * nh : 512 * (nh + 1)], in_=ps_o[:]
                )
                # Transpose each 512-col half as soon as its copy lands, so
                # the tail only waits for the last half, not the full row.
                # Pair 7's h1 halves go out on the scalar queue so the four
                # tail transposes issue two-abreast instead of serially.
                tq = nc.scalar if (p == NPAIR - 1 and hh == 1) else nc.sync
                tq.dma_start_transpose(
                    ott_tiles[(p, hh)][:, 4 * nh : 4 * (nh + 1), :],
                    oT[:, 512 * nh : 512 * (nh + 1)],
                )

        def emit_out_step(p, s):
            """outT accumulation for pair p (standard spread: one (hh, nh)
            chain of 8 chunk matmuls spans two steps through psO)."""
            k = s // 2
            hh, nh = k // 2, k % 2
            cs = list(range(4 * (s % 2), 4 * (s % 2) + 4))
            emit_out_chain_piece(p, hh, nh, cs, psO, "psO")

        def emit_norm(p, hh):
            """Normalize the transposed output oTT[(p, hh)] into out_pair(p):
            one reciprocal + one broadcast multiply over all 8 q-chunks."""
            oTT = ott_tiles[(p, hh)]
            out_pair = outp_tiles[p]
            rec = small_pool.tile([P, NT], F32, tag="rec", name=f"rec{p}_{hh}")
            nc.vector.reciprocal(rec[:], oTT[:, :, DK])
            rec_b = rec[:, :].unsqueeze(2).broadcast_to([P, NT, DK])
            nc.vector.scalar_tensor_tensor(
                out=out_pair[:, :, DK * hh : DK * (hh + 1)],
                in0=oTT[:, :, 0:DK],
                scalar=1.0,
                in1=rec_b,
                op0=mybir.AluOpType.mult,
                op1=mybir.AluOpType.mult,
            )

        def emit_out_dma(p):
            # GpSimd queue (idle): keeps regular DMAs off the sync queue,
            # which carries the xbar transposes (Tile serializes DMA-mode
            # flips on the same path).
            nc.gpsimd.dma_start(
                out=out_view[:, :, P * p : P * (p + 1)], in_=outp_tiles[p][:]
            )

        # ---- PE warm-up: the HAM clock gate keeps the PE at 1.2 GHz until
        # it has seen ~3.4us of sustained matmul activity.  The prologue is
        # DMA-paced, so without this the pair-0 chains (and early phase 0)
        # run at half clock.  Hammer a dummy tile while the DMAs stream.
        warm = big.tile([P, 512], BF16)
        nc.vector.memset(warm[:], 0.0)
        warm_n = [0]

        def emit_warm(n):
            """Dummy matmuls that keep the PE (and the HAM activity window)
            busy while the prologue is DMA-paced."""
            for _ in range(n):
                wps = psO.tile([P, 512], F32, tag="psO",
                               name=f"warm{warm_n[0]}")
                warm_n[0] += 1
                nc.tensor.matmul(wps[:], lhsT=warm[:, 0:P], rhs=warm[:],
                                 start=True, stop=True)

        emit_warm(11)

        # ---- prologue: load inputs (already bf16 + transposed on host).
        # The sync HWDGE queue is the fast one (~0.65us per 256KB piece once
        # the ~5us first-DMA latency is paid; the scalar queue is ~4.5us per
        # piece).  All input DMA is front-loaded so the main body stays
        # DMA-light (sustained queue activity throttles the whole chip by
        # ~1.2x).  xt streams per-chunk so the pair-0 chains start early.
        nc.scalar.dma_start(out=wq_sb[:, 0], in_=wq_d[:, 0])
        nc.scalar.dma_start(out=wq_sb[:, 1:2], in_=wq_d[:, 1:2])
        nc.scalar.dma_start(out=wq_sb[:, 2:4], in_=wq_d[:, 2:4])
        nc.scalar.dma_start(out=wq_sb[:, 4:8], in_=wq_d[:, 4:8])
        nc.sync.dma_start(out=wk_sb[:, 0], in_=wk_d[:, 0])
        for kd in range(NT):
            nc.sync.dma_start(out=xT[:, kd], in_=xt_d[:, kd])
        nc.sync.dma_start(out=wv_sb[:, 0:4], in_=wv_d[:, 0:4])
        nc.sync.dma_start(out=wv_sb[:, 4:8], in_=wv_d[:, 4:8])
        nc.sync.dma_start(out=wk_sb[:, 1:4], in_=wk_d[:, 1:4])
        nc.sync.dma_start(out=wk_sb[:, 4:8], in_=wk_d[:, 4:8])

        def emit_v_step(j):
            """V projection for m-tile j as two 8-MM half-chains ping-ponging
            the psO slots (free until out(0) starts in phase 1), so copies
            overlap the next half-chain."""
            for nh in range(2):
                ps = psO.tile([P, 512], F32, tag="psO", name=f"vchain{j}_{nh}")
                for kd in range(NT):
                    nc.tensor.matmul(
                        ps[:],
                        lhsT=xT[:, kd, P * j : P * (j + 1)],
                        rhs=wv_sb[:, kd, 512 * nh : 512 * (nh + 1)],
                        start=(kd == 0),
                        stop=(kd == NT - 1),
                    )
                nc.vector.tensor_copy(
                    out=vo[:, j, 8 * nh : 8 * (nh + 1), 0:DK],
                    in_=ps.rearrange("p (h v) -> p h v", v=DK),
                )
            nc.vector.memset(vo[:, j, :, DK : DK + 1], 1.0)

        # ---- pair-0 projections, kd-major so each matmul starts as soon as
        # its xT chunk lands.  Q's two halves run first (scores need all of
        # QT), K's halves follow (scores step s only needs KT chunk s, and
        # the nh0 copy covers chunks 0-3).
        prj = {}
        for which in range(4):  # 0: Qnh0, 1: Qnh1, 2: Knh0, 3: Knh1
            pool, tag = (psS, "psS") if which < 2 else (psP, "psP")
            prj[which] = pool.tile([P, 512], F32, tag=tag, name=f"prj0_{which}")
        qt_tiles[0] = qk_pool.tile([P, W], BF16, tag="qt", name="qt0")
        kt_tiles[0] = qk_pool.tile([P, W], BF16, tag="kt", name="kt0")
        for kd in range(NT):
            if kd < 6:
                # Fillers only inside the DMA-paced stretch: they keep the
                # HAM clock gate warm (4 chain MMs per ~1us xt piece is
                # under 50% duty) without delaying the post-DMA chain tail.
                emit_warm(2)
            for which in range(4):
                w_sb = wq_sb if which < 2 else wk_sb
                nh = which % 2
                nc.tensor.matmul(
                    prj[which][:],
                    lhsT=w_sb[:, 0, kd, :],
                    rhs=xT[:, kd, 512 * nh : 512 * (nh + 1)],
                    start=(kd == 0),
                    stop=(kd == NT - 1),
                )
        for which in range(4):
            dst = qt_tiles[0] if which < 2 else kt_tiles[0]
            nh = which % 2
            if nh == 1:
                # All four chains finish together (they all need the last xt
                # piece), so the copies gate scores(0).  ScalarE is idle
                # until the first exp — drain them two-abreast.
                nc.scalar.copy(
                    out=dst[:, 512 : 1024], in_=prj[which][:]
                )
            else:
                nc.vector.tensor_copy(
                    out=dst[:, 0 : 512], in_=prj[which][:]
                )

        # ---- main pipeline over phases ----
        for ph in range(NPAIR):
            last = ph == NPAIR - 1
            attn_tiles[ph] = attn_pool.tile(
                [P, NT, 2, W], BF16, tag="attnT", name=f"attnT{ph}"
            )
            if ph >= 1:
                outp_tiles[ph - 1] = outp_pool.tile(
                    [P, NT, P], F32, tag="outp", name=f"outp{ph - 1}"
                )
            if last:
                outp_tiles[ph] = outp_pool.tile(
                    [P, NT, P], F32, tag="outp", name=f"outp{ph}"
                )
            for s in range(NT):
                emit_scores_step(ph, s)
                if ph == 0 and s >= 1:
                    emit_v_step(s - 1)
                    if s == NT - 1:
                        emit_v_step(s)
                if 1 <= ph < NPAIR - 1:
                    emit_out_step(ph - 1, s)
                if last:
                    # out(6) ping-pongs psO/psP in the first two steps (all
                    # its attnT is ready); out(7)'s h0 chains then trail
                    # scores(7) chunk-by-chunk on both slots (proj(8) does
                    # not exist, so psP is free).  Pieces are only emitted
                    # once their ACT producer has been emitted.
                    if s < 2:
                        emit_out_chain_piece(NPAIR - 2, s, 0,
                                             list(range(NT)), psO, "psO")
                        emit_out_chain_piece(NPAIR - 2, s, 1,
                                             list(range(NT)), psP, "psP")
                    elif s == 2:
                        emit_out_chain_piece(NPAIR - 1, 0, 0, [0, 1, 2], psO, "psO")
                        emit_out_chain_piece(NPAIR - 1, 0, 1, [0, 1, 2], psP, "psP")
                    else:
                        emit_out_chain_piece(NPAIR - 1, 0, 0, [s], psO, "psO")
                        emit_out_chain_piece(NPAIR - 1, 0, 1, [s], psP, "psP")
                    if s == 7:
                        emit_out_chain_piece(NPAIR - 1, 1, 0,
                                             list(range(NT)), psO, "psO")
                        emit_out_chain_piece(NPAIR - 1, 1, 1,
                                             list(range(NT)), psP, "psP")
                if not last:
                    emit_proj_steps(ph + 1, s)
                # Norms: (p, hh) as soon as its transpose has been emitted,
                # spread over later phases' steps.
                if ph >= 2 and s == 1:
                    emit_norm(ph - 2, 1)
                if ph >= 1 and not last and s == 6:
                    emit_norm(ph - 1, 0)
                if last and s == 4:
                    emit_norm(NPAIR - 2, 0)
                if last and s == 5:
                    emit_norm(NPAIR - 2, 1)
            if ph >= 2:
                emit_out_dma(ph - 2)
            if last:
                # Tail: pair 6, then pair 7 (h0 norm can start as soon as
                # both its transpose halves land, overlapping the h1 chains).
                emit_out_dma(NPAIR - 2)
                emit_norm(NPAIR - 1, 0)
                emit_norm(NPAIR - 1, 1)
                emit_out_dma(NPAIR - 1)

    print(f"[kernel] trace+schedule took {_time.time() - _tb:.1f}s", flush=True)
    _t0 = _time.time()
    nc.compile()
    print(f"[kernel] bacc compile took {_time.time() - _t0:.1f}s", flush=True)
    return nc


_NC_CACHE = None


def _get_nc():
    global _NC_CACHE
    if _NC_CACHE is None:
        _NC_CACHE = build_nc()
    return _NC_CACHE


def _marshal_w(w):
    """(D, H*DK) f32 -> (P, NT, H*DK) bf16 with w[pp, dt, n] = W[128*dt+pp, n]."""
    import ml_dtypes

    w = np.asarray(w, dtype=np.float32).reshape(NT, P, H * DK)
    return np.ascontiguousarray(w.transpose(1, 0, 2)).astype(ml_dtypes.bfloat16)


def _marshal_w_pairmajor(w):
    """(D, H*DK) f32 -> (P, NPAIR, NT, P) bf16 with
    w[pp, pr, dt, c] = W[128*dt+pp, 128*pr+c]."""
    import ml_dtypes

    w = np.asarray(w, dtype=np.float32).reshape(NT, P, NPAIR, P)
    return np.ascontiguousarray(w.transpose(1, 2, 0, 3)).astype(ml_dtypes.bfloat16)


def kernel(inputs, Wq, Wk, Wv, _trace=False):
    import ml_dtypes

    inputs = np.asarray(inputs, dtype=np.float32)
    wq_m = _marshal_w_pairmajor(Wq)
    wk_m = _marshal_w_pairmajor(Wk)
    wv_m = _marshal_w(Wv)
    nc = _get_nc()
    in_maps = []
    for b in range(B):
        xt = inputs[b].T.reshape(NT, P, W)  # [dt, pp, q]
        xt = np.ascontiguousarray(xt.transpose(1, 0, 2)).astype(ml_dtypes.bfloat16)
        in_maps.append({"xt": xt, "wq": wq_m, "wk": wk_m, "wv": wv_m})
    try:
        res = run_bass_kernel_spmd(
            nc, in_maps, core_ids=list(range(B)), trace=_trace
        )
    except Exception:
        # A crashed prior session can leave the device in an unrecoverable
        # state for one execution; a single retry clears it.
        res = run_bass_kernel_spmd(
            nc, in_maps, core_ids=list(range(B)), trace=_trace
        )
    out = np.stack([np.asarray(res.results[b]["out"]) for b in range(B)])
    out = out.reshape(B, W, H, DK).astype(np.float32)
    if _trace:
        return out, res
    return out

